# revision 7
# baseline (speedup 1.0000x reference)
"""Trainium2 Bass kernel for nn_BayerFeatureExtractor (v4).

Input:  bayer [4, 1, 768, 768] f32.  Output: [4, 30, 768, 768] f32.

Sharding: pure data-parallel over 8 cores: core i handles batch b = i//2,
row-half h = i%2 (output rows [h*384, (h+1)*384)).

v4 vs v3: processes both 384-col halves of a 96-row strip together.
Horizontal maps computed once at full 776 width; matmuls write per-half
slots of 2-bank PSUM pair tiles (bufs=4 = all 8 banks); elementwise runs
once at pair width (wide activations straddle the pair slots).  dct via
rank-1 separable chain; hessian-deviatoric scale folded into lhs; fills
lhs-merged 16->12; squares taken from SBUF bf16 copies (DVE 2x mode)
instead of PSUM; engine mix tuned so PE/Act/DVE land ~70% each and Pool
holds only shallow-dep prefetch ops.  Matmul count 744 -> ~590 but
elementwise instruction count roughly halved and all engines balanced.
"""
import math
import os
import sys
from contextlib import ExitStack

import numpy as np

for _p in ('/opt/trn_rl_repo', '/root/.axon_site/_ro/trn_rl_repo'):
    if os.path.isdir(_p) and _p not in sys.path:
        sys.path.insert(0, _p)

import concourse.bass as bass
import concourse.bacc as bacc
import concourse.mybir as mybir
import concourse.tile as tile
from concourse.bass_utils import run_bass_kernel_spmd

F32 = mybir.dt.float32
BF16 = mybir.dt.bfloat16
AL = mybir.AluOpType
AF = mybir.ActivationFunctionType

EPS = 1e-6
K1, M1 = 104, 100
M2 = 96

# ----------------------------------------------------------------------------
# constants (identical math to reference._build_kernels)
# ----------------------------------------------------------------------------


def _norm(k):
    k = k - k.mean()
    return (k / max(float(np.abs(k).sum()), 1e-6)).astype(np.float32)


def _gabor(theta, size=5, sigma=1.1, wavelength=3.0, gamma=0.65):
    r = size // 2
    c = np.arange(-r, r + 1, dtype=np.float32)
    yy, xx = np.meshgrid(c, c, indexing='ij')
    xt = xx * math.cos(theta) + yy * math.sin(theta)
    yt = -xx * math.sin(theta) + yy * math.cos(theta)
    env = np.exp(-(xt ** 2 + (gamma * yt) ** 2) / (2.0 * sigma * sigma))
    return _norm(env * np.cos(2.0 * math.pi * xt / wavelength))


def build_k3():
    f32 = np.float32
    return np.stack([
        _norm(np.array([[-1, 0, 1], [-2, 0, 2], [-1, 0, 1]], f32)),
        _norm(np.array([[-1, -2, -1], [0, 0, 0], [1, 2, 1]], f32)),
        _norm(np.array([[-2, -1, 0], [-1, 0, 1], [0, 1, 2]], f32)),
        _norm(np.array([[0, 1, 2], [-1, 0, 1], [-2, -1, 0]], f32)),
    ])


def banded1(col5, off=2, M=M1):
    B = np.zeros((K1, M), np.float32)
    for x in range(M):
        for dy in range(-2, 3):
            k = x + off + dy
            if 0 <= k < K1:
                B[k, x] = col5[dy + 2]
    return B


def banded2(col5, scale=1.0):
    B = np.zeros((K1, M2), np.float32)
    for y in range(M2):
        for dy in range(-2, 3):
            k = y + 2 + dy
            if 0 <= k < 100:
                B[k, y] = col5[dy + 2] * scale
    return B


def pad5(col3):
    z = np.zeros(5, np.float32)
    z[1:4] = np.asarray(col3, np.float32)
    return z


def build_patterns_v():
    t5 = np.array([1, 2, 3, 2, 1], np.float32) / 9.0

    def mfun(ch, rp, cp):
        return {
            'r': float(rp == 1 and cp == 0),
            'b': float(rp == 0 and cp == 1),
            'gr': float(rp == 1 and cp == 1),
            'gb': float(rp == 0 and cp == 0),
            'g': float((rp == 1 and cp == 1) or (rp == 0 and cp == 0)),
        }[ch]

    V = {}
    for ch in ['r', 'b', 'g', 'gr', 'gb']:
        v = np.zeros((2, 2), np.float32)
        for rp in range(2):
            for cp in range(2):
                d = sum(t5[dy + 2] * t5[dx + 2]
                        * mfun(ch, (rp + dy) % 2, (cp + dx) % 2)
                        for dy in range(-2, 3) for dx in range(-2, 3))
                v[rp, cp] = 1.0 / max(d, EPS)
        V[ch] = v
    return V


def build_lhs1():
    k3 = build_k3()
    s = np.sin(2.0 * math.pi * np.arange(5, dtype=np.float32) / 5.0)
    c = np.cos(2.0 * math.pi * np.arange(5, dtype=np.float32) / 5.0)
    alt = np.array([1, -1, 1, -1, 1], np.float32)
    ones5 = np.ones(5, np.float32)
    sn = 5.0 * float(np.abs(s).sum())
    cn = 5.0 * float(np.abs(c).sum())
    f5 = np.cos(math.pi * (np.arange(5, dtype=np.float32) + 0.5) * 2.0 / 5.0)
    dnorm = float(np.abs(f5).sum()) ** 2
    ha = np.array([-0.25, 0.5, 0.5, 0.5, -0.25], np.float32)
    t5 = np.array([1, 2, 3, 2, 1], np.float32) / 9.0
    r2 = 1.0 / math.sqrt(2.0)

    mats, idx = [], {}

    def add(name, m):
        idx[name] = len(mats)
        mats.append(m.astype(np.float32))

    # --- ext verticals (off=2, M=100)
    add('gxV', banded1(pad5([1, 2, 1]) / 8.0))
    add('gyV', banded1(pad5([-1, 0, 1]) / 8.0))
    add('E1', banded1(pad5([-1, 0, 1]) / 8.0))
    add('E1n', banded1(pad5([1, 0, -1]) / 8.0))
    add('E2', banded1(pad5([1, 1, 1]) / 8.0))
    sumk = (k3[0] + k3[1] + k3[2] + k3[3]) * 0.25   # fold msq scale
    for dx in range(-1, 2):
        add(f'sumd{dx}', banded1(pad5(sumk[:, dx + 1])))

    # fills merged (12)
    V = build_patterns_v()
    kk = (np.arange(K1) % 2).astype(np.float32)
    Vo = banded1(t5 / 9.0) * kk[:, None]
    Ve = banded1(t5 / 9.0) * (1 - kk)[:, None]
    xpar = np.arange(M1) % 2

    def scl(base, vcol, f=1.0):
        return base * (f * vcol[xpar])[None, :]

    add('fA', scl(Vo, V['r'][:, 0]) + scl(Ve, V['g'][:, 0], -1.0))
    add('fB', scl(Vo, V['g'][:, 0], -2.0))
    add('fC', scl(Vo, V['r'][:, 1], 2.0) + scl(Ve, V['g'][:, 1], -2.0))
    add('fD', scl(Vo, V['g'][:, 1], -1.0))
    add('fE', scl(Ve, V['b'][:, 0], 2.0) + scl(Vo, V['g'][:, 0], -2.0))
    add('fF', scl(Ve, V['g'][:, 0], -1.0))
    add('fG', scl(Ve, V['b'][:, 1], 1.0) + scl(Vo, V['g'][:, 1], -1.0))
    add('fH', scl(Ve, V['g'][:, 1], -2.0))
    add('fI', scl(Vo, V['gr'][:, 0], 2.0))
    add('fJ', scl(Ve, V['gb'][:, 0], -1.0))
    add('fK', scl(Vo, V['gr'][:, 1], 1.0))
    add('fL', scl(Ve, V['gb'][:, 1], -2.0))

    # --- central verticals (off=4, M=96)
    def c_(col5):
        return banded1(col5, off=4, M=M2)

    IC = c_(pad5([0, 1, 0]))
    hyyV = c_(pad5([1, -2, 1]))
    add('hyyV', hyyV)
    add('IC', IC)
    add('nhyyVs', -hyyV * 0.5)
    add('ICs', IC * 0.5)
    add('IC25', 0.25 * IC)
    add('nhaV', c_(-ha))
    add('ICmGC', IC - c_(pad5([1, 0, 1]) * 0.25))
    add('ICn025', -0.25 * IC)
    add('hxyV', c_(pad5([-1, 0, 1]) / 4.0))
    add('hfV', c_(pad5([1, -2, 1]) / 16.0))
    add('cbV1', c_(alt / 24.96))
    add('cbV2', c_(ones5 * (-0.04 / 24.96)))
    add('shV', c_(ones5 * (0.8 / 24.0)))
    add('svV', c_((alt - 0.2) / 24.0))
    add('sinxV', c_(ones5 * (float(s[1]) / sn)))
    add('sinyV', c_(s / sn))
    add('phyV', c_(c / cn))
    phx_k = np.tile(c, (5, 1)) / cn      # kernel[dy, dx] = c[dx]/cn
    for dx in range(-2, 3):
        add(f'phx_{dx}', c_(np.full(5, phx_k[0, dx + 2], np.float32)))
    add('dctV', c_(f5 / dnorm))
    for dx in range(-2, 3):
        add(f'j_{dx}', banded2(t5, t5[dx + 2]))
        add(f'jh_{dx}', banded2(t5, t5[dx + 2] * 0.5))
    add('boxV', banded2(ones5 / 5.0, 1.0 / 5.0))
    g45 = _gabor(math.pi / 4.0)
    g135 = _gabor(3.0 * math.pi / 4.0)
    for dx in range(-2, 3):
        add(f'g45_{dx}', c_(g45[:, dx + 2]))
        add(f'g135_{dx}', c_(g135[:, dx + 2]))

    packed = np.zeros((len(mats), K1, M1), np.float32)
    for i, m in enumerate(mats):
        packed[i, :m.shape[0], :m.shape[1]] = m
    return packed, idx


def build_rowsign(h):
    sg = np.ones((4, M1), np.float32)
    for t in range(4):
        for x in range(M1):
            r = h * 384 + 96 * t - 2 + x
            if r < 0 or r >= 768:
                sg[t, x] = -1.0
    return sg.T.copy()  # [100, 4]


def build_gm():
    pp = np.arange(128)[:, None] % 2
    mm_ = np.arange(776)[None, :] % 2
    return (pp == mm_).astype(np.float32)  # [128, 776]


def build_maskch():
    er = (np.arange(768) % 2 == 0).astype(np.float32)[:, None]
    ec = (np.arange(768) % 2 == 0).astype(np.float32)[None, :]
    r_m = (1 - er) * ec
    gb_m = er * ec
    gr_m = (1 - er) * (1 - ec)
    b_m = er * (1 - ec)
    return np.stack([r_m, gr_m + gb_m, b_m])  # [3, 768, 768]


# stage central slots: ch -> slot  [96, 15, 768]
SLOT = {5: 0, 6: 1, 7: 2, 8: 3, 10: 4, 17: 5, 18: 6, 20: 7, 21: 8,
        22: 9, 23: 10, 26: 11, 29: 12, 27: 13, 28: 14}
# W ext slots: ch -> slot  [100, 10, 2, 388]
WSLOT = {0: 0, 1: 1, 2: 2, 3: 3, 4: 4, 19: 5, 9: 6, 14: 7, 15: 8, 16: 9}


# ----------------------------------------------------------------------------
# kernel builder
# ----------------------------------------------------------------------------

def build_nc():
    lhs1_np, idx = build_lhs1()
    n1 = lhs1_np.shape[0]

    nc = bacc.Bacc(None, target_bir_lowering=False)
    bayer_d = nc.dram_tensor('bayer_pad', [392, 776], BF16,
                             kind='ExternalInput')
    lhs1_d = nc.dram_tensor('lhs1', [K1, n1 * M1], BF16, kind='ExternalInput')
    rsg_d = nc.dram_tensor('rowsgn', [M1, 4], F32, kind='ExternalInput')
    gm_d = nc.dram_tensor('gm', [128, 776], BF16, kind='ExternalInput')
    out_d = nc.dram_tensor('out', [30, 384, 768], BF16, kind='ExternalOutput')

    with tile.TileContext(nc) as tc, ExitStack() as ctx:
        cpool = ctx.enter_context(tc.tile_pool(name='const', bufs=1))
        inpool = ctx.enter_context(tc.tile_pool(name='inp', bufs=2))
        hpool = ctx.enter_context(tc.tile_pool(name='hmaps', bufs=2))
        h1pool = ctx.enter_context(tc.tile_pool(name='hm1', bufs=1))
        epool = ctx.enter_context(tc.tile_pool(name='extw', bufs=1))
        mpool = ctx.enter_context(tc.tile_pool(name='cenw', bufs=1))
        spool = ctx.enter_context(tc.tile_pool(name='stage', bufs=2))
        pspool = ctx.enter_context(
            tc.tile_pool(name='ps', bufs=4, space='PSUM'))

        epsT = cpool.tile([128, 1], F32, tag='epsT', name='epsT')
        eps4T = cpool.tile([128, 1], F32, tag='eps4T', name='eps4T')
        nc.vector.memset(epsT[:], EPS)
        nc.vector.memset(eps4T[:], 4.0 * EPS)
        lhs1_t = cpool.tile([K1, n1 * M1], BF16, tag='lhs1')
        rsg_t = cpool.tile([M1, 4], F32, tag='rsg')
        gm_t = cpool.tile([128, 776], BF16, tag='gm')
        nc.sync.dma_start(lhs1_t[:, 0:2000], lhs1_d[:, 0:2000])
        nc.sync.dma_start(lhs1_t[:, 5400:6400], lhs1_d[:, 5400:6400])
        nc.sync.dma_start(lhs1_t[:, 2000:5400], lhs1_d[:, 2000:5400])
        nc.sync.dma_start(rsg_t[:], rsg_d[:])
        nc.sync.dma_start(gm_t[:], gm_d[:])

        def L(name, M=M2):
            i = idx[name]
            return lhs1_t[:, i * M1:i * M1 + M]

        def Lj(name):
            i = idx[name]
            return lhs1_t[0:100, i * M1:i * M1 + M2]

        def MM(ps, lh, rh, start, stop):
            nc.tensor.matmul(ps, lh, rh, start=start, stop=stop)

        def act(out, in_, func, bias=0.0, scale=1.0):
            if (isinstance(bias, float) and bias != 0.0
                    and func != AF.Copy):
                bt = eps4T if bias == 4.0 * EPS else epsT
                bias = bt[0:out.shape[0], :]
            nc.scalar.activation(out, in_, func, bias=bias, scale=scale)

        def P2(pp, N=384):
            return pp.rearrange('p (s n) -> p s n', s=2)[:, :, 0:N]

        # ------------------------------------------------------------------
        def hphase(t):
            BTF = inpool.tile([K1, 776], BF16, tag='BT', name='BT')
            nc.scalar.dma_start(BTF[:], bayer_d[96 * t:96 * t + 104, :])

            def H(tag, pool=hpool):
                return pool.tile([K1, 776], BF16, tag=tag, name=tag)

            BT0 = BTF[:, 2:774]
            W_ = 772
            h = {}
            a5 = H('a5', h1pool)
            nc.vector.tensor_add(a5[:, 0:W_], BTF[:, 0:772], BTF[:, 4:776])
            h['b5'] = H('b5')
            nc.vector.tensor_add(h['b5'][:, 0:W_], BTF[:, 1:773],
                                 BTF[:, 3:775])
            b5 = h['b5'][:, 0:W_]
            h['sob'] = H('sob')
            nc.vector.tensor_sub(h['sob'][:, 0:W_], BTF[:, 3:775],
                                 BTF[:, 1:773])
            h['o3'] = H('o3')
            nc.vector.tensor_add(h['o3'][:, 0:W_], b5, BT0)
            o3 = h['o3'][:, 0:W_]
            t3 = H('tsc', h1pool)
            nc.vector.tensor_scalar(t3[:, 0:W_], BT0, 3.0, None, AL.mult)
            h['t5e'] = H('t5e')
            nc.vector.tensor_add(h['t5e'][:, 0:W_], a5[:, 0:W_], t3[:, 0:W_])
            h['121'] = H('h121')
            nc.gpsimd.tensor_add(h['121'][:, 0:W_], o3, BT0)
            h['1m21'] = H('m121')
            nc.vector.tensor_sub(h['1m21'][:, 0:W_], o3, t3[:, 0:W_])
            ta = H('ta', h1pool)
            nc.gpsimd.tensor_add(ta[:, 0:W_], a5[:, 0:W_], BT0)
            h['alt'] = H('alt')
            nc.gpsimd.tensor_sub(h['alt'][:, 0:W_], ta[:, 0:W_], b5)
            h['o5'] = H('o5')
            nc.gpsimd.tensor_add(h['o5'][:, 0:W_], ta[:, 0:W_], b5)
            u1s = H('u1s', h1pool)
            nc.vector.tensor_sub(u1s[:, 0:W_], BTF[:, 1:773], BTF[:, 4:776])
            u2s = H('u2s', h1pool)
            nc.vector.tensor_sub(u2s[:, 0:W_], BT0, BTF[:, 3:775])
            ws_ = H('tsc', h1pool)
            nc.vector.tensor_scalar(ws_[:, 0:W_], u2s[:, 0:W_], 0.6183,
                                    None, AL.mult)
            h['s'] = H('hs_')
            nc.vector.tensor_add(h['s'][:, 0:W_], u1s[:, 0:W_], ws_[:, 0:W_])
            w2 = H('tsc', h1pool)
            nc.vector.tensor_scalar(w2[:, 0:W_], b5, 1.5, None, AL.mult)
            h['sh'] = H('hsh')
            nc.vector.tensor_sub(h['sh'][:, 0:W_], ta[:, 0:W_], w2[:, 0:W_])
            g2 = H('tsc', h1pool)
            nc.vector.tensor_scalar(g2[:, 0:W_], o3, 2.0, None, AL.mult)
            h['ghh'] = H('ghh')
            nc.vector.tensor_sub(h['ghh'][:, 0:W_], g2[:, 0:W_], a5[:, 0:W_])
            # dct horizontal: taps f = [.809,-.309,-1,-.309,.809]
            xc = H('xx', h1pool)
            nc.vector.tensor_scalar(xc[:, 0:W_], b5, 0.38196601, None,
                                    AL.mult)
            w1d = H('w1d', h1pool)
            nc.vector.tensor_sub(w1d[:, 0:W_], a5[:, 0:W_], xc[:, 0:W_])
            xd = H('xx', h1pool)
            nc.vector.tensor_scalar(xd[:, 0:W_], w1d[:, 0:W_], 0.80901699,
                                    None, AL.mult)
            h['dcth'] = H('dcth')
            nc.vector.tensor_sub(h['dcth'][:, 0:W_], xd[:, 0:W_], BT0)
            return BTF, h

        # ------------------------------------------------------------------
        def tphase(t, BTF, h, nxt):
            r0 = 96 * t

            def ps_new(tag):
                return pspool.tile([M1, 1024], F32, tag='pp', name=tag)

            def et(tag, pool=epool):
                return pool.tile([M1, 2, 388], BF16, tag=tag, name=tag)

            def ct(tag, w=768):
                return mpool.tile([M2, w], BF16, tag=tag, name=tag)

            def ct2(tag):
                return mpool.tile([M2, 2, 768], BF16, tag=tag, name=tag)

            stg_t = spool.tile([96, 15, 768], BF16, tag='stage')
            W_t = spool.tile([100, 10, 2, 388], BF16, tag='wext')

            def WS(ch):
                return W_t[:, WSLOT[ch], :, :]

            def stg(ch):
                return stg_t[:, SLOT[ch], :]

            def hx(name, c):        # ext rhs slice of h-map
                return h[name][:, 384 * c:384 * c + 388]

            def hc(name, c):        # central rhs slice of h-map
                return h[name][:, 384 * c + 2:384 * c + 386]

            def bx(c, dx):          # ext BT rhs
                return BTF[:, 384 * c + 2 + dx:384 * c + 390 + dx]

            def bc(c, dx):          # central BT rhs
                return BTF[:, 384 * c + 4 + dx:384 * c + 388 + dx]

            D = nc.vector
            G = nc.gpsimd

            # ---------- BT-only series
            psd = ps_new('psd')
            for c in (0, 1):
                sl = psd[:, 512 * c:512 * c + 388]
                for i, dx in enumerate((-1, 0, 1)):
                    MM(sl, L(f'sumd{dx}', M1), bx(c, dx), i == 0, i == 2)
            msq = et('msq')
            act(msq[:], P2(psd, 388), AF.Square)

            # ---------- phx direct 5-dx / dct via separable chain
            ppx = ps_new('ppx')
            pdc = ps_new('pdc')
            for c in (0, 1):
                for i, dx in enumerate(range(-2, 3)):
                    MM(ppx[:96, 512 * c:512 * c + 384], L(f'phx_{dx}'),
                       bc(c, dx), i == 0, i == 4)
                MM(pdc[:96, 512 * c:512 * c + 384], L('dctV'), hc('dcth', c),
                   True, True)
            px2 = ct('px2')
            act(px2[:], P2(ppx[:96]), AF.Square)
            d2 = ct('d2')
            act(d2[:], P2(pdc[:96]), AF.Square)

            # ---------- gx / gy
            pgx = ps_new('pgx')
            pgy = ps_new('pgy')
            for c in (0, 1):
                MM(pgx[:, 512 * c:512 * c + 388], L('gxV', M1),
                   hx('sob', c), True, True)
                MM(pgy[:, 512 * c:512 * c + 388], L('gyV', M1),
                   hx('121', c), True, True)
            D.tensor_copy(WS(0), P2(pgx, 388))
            D.tensor_copy(WS(1), P2(pgy, 388))
            sqx = et('sqx')
            D.tensor_mul(sqx[:], WS(0), WS(0))
            sqy = et('sqy')
            D.tensor_mul(sqy[:], WS(1), WS(1))
            gxy = et('gxy')
            D.tensor_mul(gxy[:], WS(0), WS(1))
            d2s = et('d2s')
            D.tensor_sub(d2s[:], sqx[:], sqy[:])
            sc_t = epool.tile([M1, 2, 2, 388], BF16, tag='sc', name='sc')
            D.tensor_add(sc_t[:, 0], sqx[:], sqy[:])       # s2s
            gxyF = et('gxyF')
            rsg = rsg_t[0:M1, t:t + 1]
            D.tensor_scalar(gxyF[:, 0, 0:2], gxy[:, 0, 0:2], rsg, -1.0,
                            AL.mult, AL.mult)
            D.tensor_scalar(gxyF[:, 0, 2:388], gxy[:, 0, 2:388], rsg,
                            None, AL.mult)
            D.tensor_scalar(gxyF[:, 1, 0:386], gxy[:, 1, 0:386], rsg,
                            None, AL.mult)
            D.tensor_scalar(gxyF[:, 1, 386:388], gxy[:, 1, 386:388], rsg,
                            -1.0, AL.mult, AL.mult)

            # ---------- fills
            prg = ps_new('prg')
            pbg = ps_new('pbg')
            pgpd = ps_new('pgpd')
            for c in (0, 1):
                t5e_ = hx('t5e', c)
                b5_ = hx('b5', c)
                ev = np.s_[:, 0::2]
                od = np.s_[:, 1::2]
                for pp_, mm_list in (
                        (prg, [('fA', t5e_, ev), ('fB', b5_, ev),
                               ('fC', b5_, od), ('fD', t5e_, od)]),
                        (pbg, [('fE', b5_, ev), ('fF', t5e_, ev),
                               ('fG', t5e_, od), ('fH', b5_, od)]),
                        (pgpd, [('fI', b5_, ev), ('fJ', t5e_, ev),
                                ('fK', t5e_, od), ('fL', b5_, od)])):
                    sl = pp_[:, 512 * c:512 * c + 388]
                    MM(sl[ev], L(mm_list[0][0], M1), mm_list[0][1][ev],
                       True, False)
                    MM(sl[ev], L(mm_list[1][0], M1), mm_list[1][1][ev],
                       False, True)
                    MM(sl[od], L(mm_list[2][0], M1), mm_list[2][1][od],
                       True, False)
                    MM(sl[od], L(mm_list[3][0], M1), mm_list[3][1][od],
                       False, True)
            act(WS(15), P2(prg, 388), AF.Copy)
            act(WS(16), P2(pbg, 388), AF.Copy)
            act(WS(14), P2(pgpd, 388), AF.Copy)
            rb2 = epool.tile([M1, 2, 2, 388], BF16, tag='rb2', name='rb2')
            D.tensor_mul(rb2[:, 0], WS(15), WS(15))
            D.tensor_mul(rb2[:, 1], WS(16), WS(16))
            D.tensor_add(sc_t[:, 1], rb2[:, 0], rb2[:, 1])  # cq
            act(W_t[:, 4:6, :, :], sc_t[:, :, :, :], AF.Sqrt, bias=EPS)

            # ---------- hessian
            plap = ps_new('plap')
            phd = ps_new('phd')
            pxy = ps_new('pxy')
            for c in (0, 1):
                sl = plap[:96, 512 * c:512 * c + 384]
                MM(sl, L('hyyV'), bc(c, 0), True, False)
                MM(sl, L('IC'), hc('1m21', c), False, True)
                sl = phd[:96, 512 * c:512 * c + 384]
                MM(sl, L('ICs'), hc('1m21', c), True, False)
                MM(sl, L('nhyyVs'), bc(c, 0), False, True)
                MM(pxy[:96, 512 * c:512 * c + 384], L('hxyV'),
                   hc('sob', c), True, True)
            D.tensor_copy(stg(5), P2(plap[:96]))
            hd2 = ct2('hd2')
            act(hd2[:, 0, :], P2(phd[:96]), AF.Square)
            act(hd2[:, 1, :], P2(pxy[:96]), AF.Square)
            hq = ct('oq')
            D.tensor_add(hq[:], hd2[:, 0, :], hd2[:, 1, :])
            hs = ct('hs')
            act(hs[:], hq[:], AF.Sqrt, bias=EPS)
            lh = ct('oq')
            D.tensor_scalar(lh[:], stg(5), 0.5, None, AL.mult)
            D.tensor_add(stg(6), lh[:], hs[:])
            D.tensor_sub(stg(7), lh[:], hs[:])

            # ---------- checker / stripes / waves
            pcb = ps_new('pcb')
            for c in (0, 1):
                sl = pcb[:96, 512 * c:512 * c + 384]
                MM(sl, L('cbV1'), hc('alt', c), True, False)
                MM(sl, L('cbV2'), hc('o5', c), False, True)
            act(stg(21), P2(pcb[:96]), AF.Abs)
            psh = ps_new('psh')
            psv = ps_new('psv')
            for c in (0, 1):
                MM(psh[:96, 512 * c:512 * c + 384], L('shV'), hc('sh', c),
                   True, True)
                MM(psv[:96, 512 * c:512 * c + 384], L('svV'), hc('o5', c),
                   True, True)
            act(stg(22), P2(psh[:96]), AF.Copy)
            act(stg(23), P2(psv[:96]), AF.Copy)
            psx = ps_new('psx')
            psy = ps_new('psy')
            for c in (0, 1):
                MM(psx[:96, 512 * c:512 * c + 384], L('sinxV'), hc('s', c),
                   True, True)
                MM(psy[:96, 512 * c:512 * c + 384], L('sinyV'), hc('o5', c),
                   True, True)
            act(stg(27), P2(psx[:96]), AF.Copy)
            act(stg(28), P2(psy[:96]), AF.Copy)
            ppy = ps_new('ppy')
            pf = ps_new('pf')
            for c in (0, 1):
                MM(ppy[:96, 512 * c:512 * c + 384], L('phyV'), hc('o5', c),
                   True, True)
                MM(pf[:96, 512 * c:512 * c + 384], L('hfV'), hc('1m21', c),
                   True, True)
            py2 = ct('py2')
            act(py2[:], P2(ppy[:96]), AF.Square)
            f2 = ct('f2')
            act(f2[:], P2(pf[:96]), AF.Square)
            ph2 = ct2('ph2')
            D.tensor_add(ph2[:, 0, :], px2[:], py2[:])
            D.tensor_add(ph2[:, 1, :], d2[:], f2[:])
            act(stg_t[:, 11:13, :], ph2[:, :, :], AF.Sqrt, bias=EPS)

            # ---------- gir / dgc
            pgir = ps_new('pgir')
            ptg = ps_new('ptg')
            for c in (0, 1):
                sl = pgir[:96, 512 * c:512 * c + 384]
                MM(sl, L('ICmGC'), bc(c, 0), True, False)
                MM(sl, L('ICn025'), hc('b5', c), False, True)
                sl = ptg[:96, 512 * c:512 * c + 384]
                MM(sl, L('IC25'), hc('ghh', c), True, False)
                MM(sl, L('nhaV'), bc(c, 0), False, True)
            gmv = gm_t[0:96, 4:772].rearrange('p (s n) -> p s n', s=2)
            D.tensor_mul(stg(17).rearrange('p (s n) -> p s n', s=2),
                         P2(pgir[:96]), gmv)
            act(stg(18), P2(ptg[:96]), AF.Abs)

            # ---------- gdm / gda / dir_var
            pu = ps_new('pu')
            pv = ps_new('pv')
            for c in (0, 1):
                sl = pu[:, 512 * c:512 * c + 388]
                MM(sl, L('E1', M1), hx('o3', c), True, False)
                MM(sl, L('E2', M1), hx('sob', c), False, True)
                sl = pv[:, 512 * c:512 * c + 388]
                MM(sl, L('E2', M1), hx('sob', c), True, False)
                MM(sl, L('E1n', M1), hx('o3', c), False, True)
            D.tensor_copy(WS(2), P2(pu, 388))
            D.tensor_copy(WS(3), P2(pv, 388))
            sq1 = et('sq1')
            D.tensor_mul(sq1[:], WS(2), WS(2))
            sq2 = et('sq2')
            D.tensor_mul(sq2[:], WS(3), WS(3))
            qa = et('qa')
            D.tensor_add(qa[:], sq1[:], sq2[:])
            qb = et('qb')
            D.tensor_add(qb[:], qa[:], sc_t[:, 0])
            tdv = et('qa')
            D.tensor_scalar(tdv[:], qb[:], 0.25, None, AL.mult)
            D.tensor_sub(WS(9), tdv[:], msq[:])

            p45 = ps_new('p45')
            p135 = ps_new('p135')
            for c in (0, 1):
                for i, dx in enumerate(range(-2, 3)):
                    MM(p45[:96, 512 * c:512 * c + 384], L(f'g45_{dx}'),
                       bc(c, dx), i == 0, i == 4)
                for i, dx in enumerate(range(-2, 3)):
                    MM(p135[:96, 512 * c:512 * c + 384], L(f'g135_{dx}'),
                       bc(c, dx), i == 0, i == 4)
            o12 = ct2('o12')
            act(o12[:, 0, :], P2(p45[:96]), AF.Square)
            act(o12[:, 1, :], P2(p135[:96]), AF.Square)
            oq = ct('oq')
            D.tensor_add(oq[:], o12[:, 0, :], o12[:, 1, :])
            act(stg(10), oq[:], AF.Sqrt, bias=EPS)

            # ---------- next strip's horizontal maps (pipelined)
            ret = hphase(nxt) if nxt is not None else None

            # ---------- J (5-dx accumulation on PE)
            jd = ps_new('jd')
            js = ps_new('js')
            jxy = ps_new('jxy')
            for c in (0, 1):
                for i, dx in enumerate(range(-2, 3)):
                    MM(jd[:96, 512 * c:512 * c + 384], Lj(f'jh_{dx}'),
                       d2s[0:100, c, 2 + dx:386 + dx], i == 0, i == 4)
                for i, dx in enumerate(range(-2, 3)):
                    MM(js[:96, 512 * c:512 * c + 384], Lj(f'j_{dx}'),
                       sc_t[0:100, 0, c, 2 + dx:386 + dx], i == 0, i == 4)
                for i, dx in enumerate(range(-2, 3)):
                    MM(jxy[:96, 512 * c:512 * c + 384], Lj(f'j_{dx}'),
                       gxyF[0:100, c, 2 + dx:386 + dx], i == 0, i == 4)
            jsq = ct2('jsq')
            act(jsq[:, 0, :], P2(jd[:96]), AF.Square)
            act(jsq[:, 1, :], P2(jxy[:96]), AF.Square)
            qj = ct('oq')
            D.tensor_add(qj[:], jsq[:, 0, :], jsq[:, 1, :])
            anum = ct('f2')
            act(anum[:], qj[:], AF.Sqrt, bias=4.0 * EPS, scale=4.0)
            smeB = ct('py2')
            act(smeB[:], P2(js[:96]), AF.Copy, bias=EPS)
            recB = ct('recB')
            with nc.allow_low_precision(reason='2e-2 output tolerance'):
                D.reciprocal(recB[:], smeB[:])
            D.tensor_mul(stg(8), anum[:], recB[:])

            # ---------- box -> cdv  (5-dx on PE; same lhs each)
            pbr = ps_new('pbr')
            pbb = ps_new('pbb')
            pbq = ps_new('pbq')
            for c in (0, 1):
                for i, dx in enumerate(range(-2, 3)):
                    MM(pbr[:96, 512 * c:512 * c + 384], Lj('boxV'),
                       W_t[0:100, WSLOT[15], c, 2 + dx:386 + dx],
                       i == 0, i == 4)
                for i, dx in enumerate(range(-2, 3)):
                    MM(pbb[:96, 512 * c:512 * c + 384], Lj('boxV'),
                       W_t[0:100, WSLOT[16], c, 2 + dx:386 + dx],
                       i == 0, i == 4)
                for i, dx in enumerate(range(-2, 3)):
                    MM(pbq[:96, 512 * c:512 * c + 384], Lj('boxV'),
                       sc_t[0:100, 1, c, 2 + dx:386 + dx], i == 0, i == 4)
            q12 = ct2('q12')
            act(q12[:, 0, :], P2(pbr[:96]), AF.Square)
            act(q12[:, 1, :], P2(pbb[:96]), AF.Square)
            u12 = ct('oq')
            D.tensor_add(u12[:], q12[:, 0, :], q12[:, 1, :])
            D.tensor_sub(stg(20).rearrange('p (s n) -> p s n', s=2),
                         P2(pbq[:96]), u12[:].rearrange(
                             'p (s n) -> p s n', s=2))

            # ---------- output DMAs
            rows = np.s_[r0:r0 + 96]
            nc.sync.dma_start(
                out_d[5:9, rows, :].rearrange('n p w -> p n w'),
                stg_t[:, 0:4, :])
            nc.sync.dma_start(out_d[10, rows, :], stg_t[:, 4, :])
            nc.sync.dma_start(
                out_d[17:19, rows, :].rearrange('n p w -> p n w'),
                stg_t[:, 5:7, :])
            nc.sync.dma_start(
                out_d[20:24, rows, :].rearrange('n p w -> p n w'),
                stg_t[:, 7:11, :])
            nc.sync.dma_start(out_d[26, rows, :], stg_t[:, 11, :])
            nc.sync.dma_start(
                out_d[27:29, rows, :].rearrange('n p w -> p n w'),
                stg_t[:, 13:15, :])
            nc.sync.dma_start(out_d[29, rows, :], stg_t[:, 12, :])
            for c in (0, 1):
                nc.sync.dma_start(
                    out_d[0:4, rows, 384 * c:384 * c + 384]
                    .rearrange('n p w -> p n w'),
                    W_t[2:98, 0:4, c, 2:386])
                nc.sync.dma_start(
                    out_d[15:17, rows, 384 * c:384 * c + 384]
                    .rearrange('n p w -> p n w'),
                    W_t[2:98, 8:10, c, 2:386])
            nc.sync.dma_start(
                out_d[4, rows, :].rearrange('p (s w) -> p s w', s=2),
                W_t[2:98, 4, :, 2:386])
            nc.sync.dma_start(
                out_d[19, rows, :].rearrange('p (s w) -> p s w', s=2),
                W_t[2:98, 5, :, 2:386])
            nc.sync.dma_start(
                out_d[9, rows, :].rearrange('p (s w) -> p s w', s=2),
                W_t[2:98, 6, :, 2:386])
            nc.sync.dma_start(
                out_d[14, rows, :].rearrange('p (s w) -> p s w', s=2),
                W_t[2:98, 7, :, 2:386])
            return ret

        cur = hphase(0)
        for t in range(4):
            cur = tphase(t, *cur, nxt=t + 1 if t < 3 else None)

    nc.compile()
    return nc, n1


_STATE = {}


def _get_state():
    if 'nc' not in _STATE:
        import ml_dtypes
        lhs1_np, _ = build_lhs1()
        n1 = lhs1_np.shape[0]
        nc, n1b = build_nc()
        assert n1 == n1b
        lhs1_pack = np.ascontiguousarray(
            lhs1_np.transpose(1, 0, 2).reshape(K1, n1 * M1))
        _STATE.update(
            nc=nc,
            lhs1=np.ascontiguousarray(lhs1_pack.astype(ml_dtypes.bfloat16)),
            gm=np.ascontiguousarray(build_gm().astype(ml_dtypes.bfloat16)),
            mch=np.ascontiguousarray(build_maskch()),
            rsg=[np.ascontiguousarray(build_rowsign(0)),
                 np.ascontiguousarray(build_rowsign(1))])
    return _STATE


def _run(bayer, trace=False, **kw):
    import ml_dtypes
    st = _get_state()
    bayer = np.ascontiguousarray(np.asarray(bayer, dtype=np.float32))
    in_maps = []
    for core in range(8):
        b, hh = core // 2, core % 2
        Pimg = np.pad(bayer[b, 0], 4, mode='reflect')
        bp = np.ascontiguousarray(
            Pimg[hh * 384:hh * 384 + 392, :].astype(ml_dtypes.bfloat16))
        in_maps.append({'bayer_pad': bp, 'lhs1': st['lhs1'],
                        'gm': st['gm'], 'rowsgn': st['rsg'][hh]})
    res = run_bass_kernel_spmd(st['nc'], in_maps, core_ids=list(range(8)),
                               trace=trace, **kw)
    out = np.empty((4, 30, 768, 768), np.float32)
    for core in range(8):
        b, hh = core // 2, core % 2
        sl = np.s_[hh * 384:(hh + 1) * 384]
        out[b, :, sl, :] = res.results[core]['out'].astype(np.float32)
    out[:, 24] = out[:, 22]
    out[:, 25] = out[:, 23]
    out[:, 11:14] = st['mch'][None]
    return out, res


def kernel(bayer):
    out, _ = _run(bayer, trace=False)
    return out
